# revision 15
# baseline (speedup 1.0000x reference)
"""Trainium2 Bass kernel for nn_MemoryEngineLayer (scatter_memory).

Contract: kernel(**inputs) takes FULL unsharded inputs (as produced by
setup_inputs()) and returns the FULL [B, T, H] output. Internally the batch
dim (B=8) is sharded across 8 NeuronCores (pure data parallelism); each core
runs the T=4096 recurrent scan for its own sequence.

Math (validated against the jax reference, rel err ~1e-8):
  m'_t   = x_t @ (beta/gamma * basis[:, :256])           # [256], top-8 inject
  inj'_t = where(|m'_t| >= kth8(|m'_t|), m'_t, 0)
  u_t    = r_{t-1} u_{t-1} + (g/gamma) r_{t-2} u_{t-2} + inj'_t   (complex 256)
  s_t    = 1 + 2 r_{t-1} <u_{t-1}, P_t> + ||P_t||^2,  P_t = g' r_{t-2} u_{t-2} + inj'_t
  r_t    = 1/sqrt(s_t + 1e-16)        # scale-invariant renorm
  y_t    = x_t + Re(r_t u_t) @ (alpha * bg * basis[:, :256]).T

On-chip layout: complex state as [128 partitions, 4 cols] = (re_lo, re_hi,
im_lo, im_hi) with slot s = q*128 + p. The per-step norm uses the identity
||r u_{t-1}|| = 1, keeping the partition_all_reduce off the r-dependency
chain.
"""

import numpy as np

H, MEM, S = 1024, 256, 272
B, T = 8, 4096
TOPK = 8
GAMMA, BETA, PTS = 0.92, 0.08, 0.4
PCH = 128  # timesteps per chunk

_program_cache = {}


def _sigmoid(v):
    return 1.0 / (1.0 + np.exp(-v.astype(np.float64)))


def _build_program(n_chunks: int, loop_reps: int = 1, pp_on_act: bool = False, fused_n: bool = True, z_on_dve: bool = False, gr_on_dve: bool = False, t_no_allred: bool = False, t_no_chain: bool = False, pe_allred: bool = True, use_rsqrt: bool = False):
    import concourse.bass as bass
    import concourse.bacc as bacc
    import concourse.mybir as mybir
    from concourse.tile import TileContext
    from concourse.masks import make_identity
    from concourse import bass_isa

    f32 = mybir.dt.float32
    Alu = mybir.AluOpType
    Act = mybir.ActivationFunctionType
    Tq = n_chunks * PCH

    from concourse._compat import get_trn_type
    nc = bacc.Bacc(get_trn_type() or "TRN2", target_bir_lowering=False, debug=False)
    xb = nc.declare_dram_parameter("xb", [Tq, H], f32, isOutput=False)
    basis_m = nc.declare_dram_parameter("basis_m", [128, 8 * 256], f32, isOutput=False)
    basis_y = nc.declare_dram_parameter("basis_y", [128, 16 * 128], f32, isOutput=False)
    v0d = nc.declare_dram_parameter("v0", [128, 4], f32, isOutput=False)
    scal = nc.declare_dram_parameter("scal", [128, 3], f32, isOutput=False)  # [ones, gp, one]
    yb = nc.declare_dram_parameter("yb", [Tq, H], f32, isOutput=True)

    with TileContext(nc) as tc:
        with (
            tc.tile_pool(name="const", bufs=1) as cpool,
            tc.tile_pool(name="xio", bufs=4) as xpool,
            tc.tile_pool(name="work", bufs=2) as wpool,
            tc.tile_pool(name="scan", bufs=4) as spool,
            tc.tile_pool(name="ps_t", bufs=2, space="PSUM") as ps_t,
            tc.tile_pool(name="ps_m", bufs=1, space="PSUM") as ps_m,
            tc.tile_pool(name="ps_y", bufs=2, space="PSUM") as ps_y,
            tc.tile_pool(name="ps_r", bufs=2, space="PSUM") as ps_r,
        ):
            # ---- constants ----
            bm_sb = cpool.tile([128, 8 * 256], f32, tag="bm")
            nc.sync.dma_start(bm_sb, basis_m[:])
            by_sb = cpool.tile([128, 16 * 128], f32, tag="by")
            nc.sync.dma_start(by_sb, basis_y[:])
            v0_sb = cpool.tile([128, 4], f32, tag="v0")
            nc.sync.dma_start(v0_sb, v0d[:])
            sc_sb = cpool.tile([128, 3], f32, tag="sc")
            nc.sync.dma_start(sc_sb, scal[:])
            ident = cpool.tile([128, 128], f32, tag="ident")
            make_identity(nc, ident[:])
            ones_row = cpool.tile([1, 128], f32, tag="ones_row")
            nc.vector.memset(ones_row[:], 1.0)
            ones_mat = cpool.tile([128, 128], f32, tag="ones_mat")
            nc.vector.memset(ones_mat[:], 1.0)

            ones_ap = sc_sb[:, 0:1]
            gp_ap = sc_sb[:, 1:2]
            one_ap = sc_sb[:, 2:3]

            # Warm-ups: walrus allows at most ONE sync wait per PE matmul, so
            # make each engine observe the constant tiles via single-dep ops
            # before any real consumer needs them together.
            warm_acc = ps_m.tile([128, 1], f32, tag="warm")

            def pe_touch(sb_slice):
                # tiny N=1 matmul whose ONLY fresh dep is sb_slice; writes to
                # the shared warm PSUM tile (same-engine WAW, no semaphore)
                nc.tensor.matmul(
                    warm_acc[:], sb_slice, ident[:, 0:1], start=True, stop=True
                )

            pe_touch(ident[:, 0:128])  # waits Pool (make_identity) only
            pe_touch(bm_sb[:, 0:128])  # waits bm DMA only
            pe_touch(by_sb[:, 0:128])  # waits by DMA only
            wsc = spool.tile([128, 4], f32, tag="wsc")
            nc.vector.tensor_copy(wsc[:], v0_sb[:])  # DVE observes v0 DMA
            wsc2 = spool.tile([128, 3], f32, tag="wsc2")
            nc.vector.tensor_copy(wsc2[:], sc_sb[:])  # DVE observes scal DMA
            wsc3 = spool.tile([128, 3], f32, tag="wsc3")
            nc.scalar.copy(wsc3[:], sc_sb[:])  # ACT observes scal DMA

            # scan state carried across chunks (python vars reference tiles)
            u1, u2 = v0_sb, v0_sb          # u_{t-1}, u_{t-2}
            r1 = ones_ap                    # r_{t-1} [128,1]
            # FIFO of gp*r values: step t pops gp*r_{t-2}, pushes gp*r_t
            gr_fifo = [gp_ap, gp_ap]        # gp*r_{-2}, gp*r_{-1}

            # P ring: 4 persistent [128,5] tiles; cols 0:4 hold P_t, col 4 is
            # the constant sqrt(1/128) so ACT Square+accum yields
            # sum(P^2) + 1/128 per partition -> +1 total after the
            # partition_all_reduce (the "1" of s = 1 + 2 r IP + ||P||^2).
            P_ring = []
            for i in range(4):
                pring_tile = cpool.tile([128, 5], f32, tag=f"Pring{i}")
                P_ring.append(pring_tile)
                nc.vector.memset(pring_tile[:, 4:5], float(np.sqrt(1.0 / 128.0)))

            from contextlib import nullcontext
            rep_ctx = tc.For_i(0, loop_reps, 1) if loop_reps > 1 else nullcontext()
            with rep_ctx:
              # (re)bind scan state at body start so each repetition restarts
              u1, u2 = v0_sb, v0_sb
              r1 = ones_ap
              gr_fifo = [gp_ap, gp_ap]
              for c in range(n_chunks):
                  t0 = c * PCH
                  x_sb = xpool.tile([128, H], f32, tag="x")
                  nc.sync.dma_start(x_sb, xb[t0 : t0 + PCH, :])

                  # transpose x chunk -> xT [h, t] blocks
                  xT_sb = xpool.tile([128, H], f32, tag="xT")
                  # single-dep touch so the real transposes don't combine the
                  # x-DMA wait with a PSUM-slot-release wait
                  pe_touch(x_sb[:, 0:128])
                  for hi in range(8):
                      tps = ps_t.tile([128, 128], f32, tag="tps")
                      nc.tensor.transpose(tps, x_sb[:, hi * 128 : (hi + 1) * 128], ident[:])
                      nc.scalar.copy(xT_sb[:, hi * 128 : (hi + 1) * 128], tps[:])

                  # m' = x @ Wm  -> psum [t=128, s=256]
                  m_ps = ps_m.tile([128, 256], f32, tag="m")
                  for hi in range(8):
                      nc.tensor.matmul(
                          m_ps[:],
                          xT_sb[:, hi * 128 : (hi + 1) * 128],
                          bm_sb[:, hi * 256 : (hi + 1) * 256],
                          start=(hi == 0),
                          stop=(hi == 7),
                      )

                  # top-8 threshold + injection
                  mag = wpool.tile([128, 256], f32, tag="mag")
                  nc.scalar.activation(mag[:], m_ps[:], Act.Abs)
                  mx8 = wpool.tile([128, 8], f32, tag="mx8")
                  nc.vector.max(mx8[:], mag[:])
                  inj = wpool.tile([128, 256], f32, tag="inj")
                  nc.vector.scalar_tensor_tensor(
                      out=inj[:], in0=mag[:], scalar=mx8[:, 7:8], in1=m_ps[:],
                      op0=Alu.is_ge, op1=Alu.mult,
                  )

                  # transpose inj -> injT [slot, t] with zero upper half (imag)
                  injT = wpool.tile([128, 512], f32, tag="injT")
                  nc.gpsimd.memset(injT[:, 256:512], 0.0)
                  for q in range(2):
                      tps = ps_t.tile([128, 128], f32, tag="tps")
                      nc.tensor.transpose(tps, inj[:, q * 128 : (q + 1) * 128], ident[:])
                      nc.scalar.copy(injT[:, q * 128 : (q + 1) * 128], tps[:])

                  z_sb = wpool.tile([128, 256], f32, tag="z")

                  # ---- the sequential scan: 128 steps ----
                  for t in range(PCH):
                      inj4 = injT[:, t : 512 : 128]  # (re_lo, re_hi, 0, 0)
                      gr2 = gr_fifo.pop(0) if not t_no_chain else gp_ap
                      P = P_ring[t % 4]
                      nc.vector.scalar_tensor_tensor(
                          out=P[:, 0:4], in0=u2[:], scalar=gr2, in1=inj4,
                          op0=Alu.mult, op1=Alu.add,
                      )
                      u = spool.tile([128, 4], f32, tag="u")
                      nc.vector.scalar_tensor_tensor(
                          out=u[:], in0=u1[:], scalar=(ones_ap if t_no_chain else r1), in1=P[:, 0:4],
                          op0=Alu.mult, op1=Alu.add,
                      )
                      # sq2 col0 = sum(2*u1*P) (DVE), col1 = sum(P^2)+1/128 (ACT)
                      # (tensor_tensor_reduce crashes the device on this runtime;
                      # stt accum_out / ACT Square accum_out are the variants
                      # that work)
                      sq2 = spool.tile([128, 2], f32, tag="sq2")
                      d0 = spool.tile([128, 4], f32, tag="d0")
                      nc.vector.scalar_tensor_tensor(
                          out=d0[:], in0=u1[:], scalar=2.0, in1=P[:, 0:4],
                          op0=Alu.mult, op1=Alu.mult, accum_out=sq2[:, 0:1],
                      )
                      d1 = spool.tile([128, 5], f32, tag="d1")
                      if pp_on_act:
                          nc.scalar.activation(
                              d1[:], P[:], Act.Square, accum_out=sq2[:, 1:2]
                          )
                      else:
                          nc.vector.scalar_tensor_tensor(
                              out=d1[:], in0=P[:], scalar=1.0, in1=P[:],
                              op0=Alu.mult, op1=Alu.mult, accum_out=sq2[:, 1:2],
                          )
                      if pe_allred and not t_no_allred:
                          # one all-ones matmul = partition collapse AND
                          # broadcast (the Pool partition_all_reduce costs
                          # ~1us/step on HW)
                          sq2r = ps_r.tile([128, 2], f32, tag="ps_b")
                          nc.tensor.matmul(
                              sq2r[:], ones_mat[:], sq2[:], start=True, stop=True
                          )
                          # PP+1 must live in SBUF (stt can't read two PSUM
                          # operands; ACT bias must be SBUF)
                          pp1_sb = spool.tile([128, 1], f32, tag="pp1")
                          nc.scalar.copy(pp1_sb[:], sq2r[:, 1:2])
                      else:
                          sq2r = spool.tile([128, 2], f32, tag="sq2r")
                          if t_no_allred:  # timing-only: wrong math
                              nc.vector.tensor_copy(sq2r[:], sq2[:])
                          else:
                              nc.gpsimd.partition_all_reduce(
                                  sq2r[:], sq2[:], 128, bass_isa.ReduceOp.add
                              )
                      n_t = spool.tile([128, 1], f32, tag="n")
                      if t_no_chain:  # timing-only: wrong math, breaks r recurrence
                          s_t = spool.tile([128, 1], f32, tag="s")
                          nc.vector.scalar_tensor_tensor(
                              out=s_t[:], in0=sq2r[:, 0:1], scalar=ones_ap, in1=sq2r[:, 1:2],
                              op0=Alu.mult, op1=Alu.add,
                          )
                          nc.scalar.activation(n_t[:], s_t[:], Act.Sqrt)
                      elif use_rsqrt and pe_allred:
                          # r = rsqrt(2*IP_tot*r1 + (PP_tot+1)) in one ACT op —
                          # removes the DVE reciprocal from the chain entirely
                          pass
                      elif fused_n and pe_allred:
                          # n = sqrt(2*IP_tot*r1 + (PP_tot+1)) in one ACT op,
                          # reading 2IP from PSUM and PP+1 from SBUF
                          nc.scalar.activation(
                              n_t[:], sq2r[:, 0:1], Act.Sqrt, scale=r1, bias=pp1_sb[:]
                          )
                      elif fused_n:
                          # n = sqrt(2*IP*r1 + (||P||^2 + 1)) in one ACT op
                          nc.scalar.activation(
                              n_t[:], sq2r[:, 0:1], Act.Sqrt, scale=r1, bias=sq2r[:, 1:2]
                          )
                      else:
                          s_t = spool.tile([128, 1], f32, tag="s")
                          nc.vector.scalar_tensor_tensor(
                              out=s_t[:], in0=sq2r[:, 0:1], scalar=r1, in1=sq2r[:, 1:2],
                              op0=Alu.mult, op1=Alu.add,
                          )
                          nc.scalar.activation(n_t[:], s_t[:], Act.Sqrt)
                      r_t = spool.tile([128, 1], f32, tag="r")
                      if use_rsqrt and pe_allred and not t_no_chain:
                          # Abs_reciprocal_sqrt == rsqrt for s>0 (4e-5 rel err
                          # measured on HW); one ACT op replaces Sqrt + DVE
                          # reciprocal, removing an edge from the r-chain
                          nc.scalar.activation(
                              r_t[:], sq2r[:, 0:1], Act.Abs_reciprocal_sqrt,
                              scale=r1, bias=pp1_sb[:],
                          )
                      else:
                          nc.vector.reciprocal(r_t[:], n_t[:])
                      gr_t = spool.tile([128, 1], f32, tag="gr")
                      if gr_on_dve:
                          nc.vector.tensor_scalar_mul(gr_t[:], r_t[:], gp_ap)
                      else:
                          nc.scalar.activation(gr_t[:], r_t[:], Act.Copy, scale=gp_ap)
                      # z_t = Re(r_t * u_t)
                      if z_on_dve:
                          nc.vector.tensor_scalar_mul(
                              z_sb[:, t : 256 : 128], u[:, 0:2], r_t[:]
                          )
                      else:
                          nc.scalar.activation(
                              z_sb[:, t : 256 : 128], u[:, 0:2], Act.Copy, scale=r_t[:]
                          )
                      u2, u1 = u1, u
                      gr_fifo.append(gr_t[:])
                      r1 = r_t[:]

                  # ---- y^T = Wy^T @ z + x^T ; transpose back; store ----
                  y_sb = xpool.tile([128, H], f32, tag="y")
                  for hi in range(8):
                      yT_ps = ps_y.tile([128, 128], f32, tag="yT")
                      nc.tensor.matmul(
                          yT_ps[:], by_sb[:, (hi * 2 + 0) * 128 : (hi * 2 + 1) * 128],
                          z_sb[:, 0:128], start=True, stop=False,
                      )
                      nc.tensor.matmul(
                          yT_ps[:], by_sb[:, (hi * 2 + 1) * 128 : (hi * 2 + 2) * 128],
                          z_sb[:, 128:256], start=False, stop=False,
                      )
                      nc.tensor.matmul(
                          yT_ps[:], ident[:], xT_sb[:, hi * 128 : (hi + 1) * 128],
                          start=False, stop=True,
                      )
                      yT_sb = wpool.tile([128, 128], f32, tag="yTs")
                      nc.scalar.copy(yT_sb[:], yT_ps[:])
                      y_ps = ps_y.tile([128, 128], f32, tag="yT")
                      nc.tensor.transpose(y_ps, yT_sb[:], ident[:])
                      nc.scalar.copy(y_sb[:, hi * 128 : (hi + 1) * 128], y_ps[:])
                  nc.sync.dma_start(yb[t0 : t0 + PCH, :], y_sb[:])

    nc.compile()
    return nc


def _build_program_v2(
    n_chunks: int,
    loop_reps: int = 1,
    r_mode: str = "abs_rsqrt",
    z_eng: str = "act",
    u_b_eng: str = "dve",
    p_b_eng: str = "dve",
    gr_eng: str = "dve",
):
    # NOTE: Pool (gpsimd) scalar_tensor_tensor/tensor_scalar ops crash the
    # walrus codegen in this toolchain — only "act"/"dve" engines are safe
    # for the scan body (gpsimd memset is fine).
    """Paired-step scan: two timesteps share one PE partition-reduce and one
    ACT round-trip.  Identity used: u_t = r_{t-1} u_{t-1} + P_t does not
    depend on r_t, so IP/PP inner products for steps (2k, 2k+1) are all
    computable before either step's renorm scalar:

      P_a = gp r_{a-2} u_{a-2} + inj_a ;  u_a = r_{a-1} u_{a-1} + P_a
      P_b = gp r_{a-1} u_{a-1} + inj_b ;  u_b = r_a u_a + P_b
      r_a = rsqrt(2<u_{a-1},P_a> r_{a-1} + ||P_a||^2 + 1)
      r_b = rsqrt(2<u_a,  P_b> r_a     + ||P_b||^2 + 1)

    One [128,4] ones-matmul reduces (2IPa, PPa+1, 2IPb, PPb+1) at once; the
    two rsqrts run back-to-back on ACT (Abs_reciprocal_sqrt, measured 4e-5
    rel err on HW, amplified ~12x by the gamma=0.92 contraction -> ~5e-4,
    well under the 2e-2 gate)."""
    import concourse.bass as bass
    import concourse.bacc as bacc
    import concourse.mybir as mybir
    from concourse.tile import TileContext
    from concourse.masks import make_identity

    f32 = mybir.dt.float32
    Alu = mybir.AluOpType
    Act = mybir.ActivationFunctionType
    Tq = n_chunks * PCH

    from concourse._compat import get_trn_type

    nc = bacc.Bacc(get_trn_type() or "TRN2", target_bir_lowering=False, debug=False)
    xb = nc.declare_dram_parameter("xb", [Tq, H], f32, isOutput=False)
    basis_m = nc.declare_dram_parameter("basis_m", [128, 8 * 256], f32, isOutput=False)
    basis_y = nc.declare_dram_parameter("basis_y", [128, 16 * 128], f32, isOutput=False)
    v0d = nc.declare_dram_parameter("v0", [128, 4], f32, isOutput=False)
    scal = nc.declare_dram_parameter("scal", [128, 3], f32, isOutput=False)
    yb = nc.declare_dram_parameter("yb", [Tq, H], f32, isOutput=True)

    P_ENG = {"pool": "gpsimd", "dve": "vector"}

    with TileContext(nc) as tc:
        with (
            tc.tile_pool(name="const", bufs=1) as cpool,
            tc.tile_pool(name="xio", bufs=4) as xpool,
            tc.tile_pool(name="work", bufs=2) as wpool,
            tc.tile_pool(name="scan", bufs=4) as spool,
            tc.tile_pool(name="ps_t", bufs=2, space="PSUM") as ps_t,
            tc.tile_pool(name="ps_m", bufs=1, space="PSUM") as ps_m,
            tc.tile_pool(name="ps_y", bufs=2, space="PSUM") as ps_y,
            tc.tile_pool(name="ps_r", bufs=2, space="PSUM") as ps_r,
        ):
            bm_sb = cpool.tile([128, 8 * 256], f32, tag="bm")
            nc.sync.dma_start(bm_sb, basis_m[:])
            by_sb = cpool.tile([128, 16 * 128], f32, tag="by")
            nc.sync.dma_start(by_sb, basis_y[:])
            v0_sb = cpool.tile([128, 4], f32, tag="v0")
            nc.sync.dma_start(v0_sb, v0d[:])
            sc_sb = cpool.tile([128, 3], f32, tag="sc")
            nc.sync.dma_start(sc_sb, scal[:])
            ident = cpool.tile([128, 128], f32, tag="ident")
            make_identity(nc, ident[:])
            ones_mat = cpool.tile([128, 128], f32, tag="ones_mat")
            nc.vector.memset(ones_mat[:], 1.0)

            ones_ap = sc_sb[:, 0:1]
            gp_ap = sc_sb[:, 1:2]

            warm_acc = ps_m.tile([128, 1], f32, tag="warm")

            def pe_touch(sb_slice):
                nc.tensor.matmul(
                    warm_acc[:], sb_slice, ident[:, 0:1], start=True, stop=True
                )

            pe_touch(ident[:, 0:128])
            pe_touch(bm_sb[:, 0:128])
            pe_touch(by_sb[:, 0:128])
            wsc = spool.tile([128, 4], f32, tag="wsc")
            nc.vector.tensor_copy(wsc[:], v0_sb[:])
            wsc2 = spool.tile([128, 3], f32, tag="wsc2")
            nc.vector.tensor_copy(wsc2[:], sc_sb[:])
            wsc3 = spool.tile([128, 3], f32, tag="wsc3")
            nc.scalar.copy(wsc3[:], sc_sb[:])
            wsc4 = spool.tile([128, 4], f32, tag="wsc4")
            nc.gpsimd.tensor_copy(wsc4[:], v0_sb[:])
            wsc5 = spool.tile([128, 3], f32, tag="wsc5")
            nc.gpsimd.tensor_copy(wsc5[:], sc_sb[:])

            # P ring: cols 0:4 = P_t, col 4 = sqrt(1/128) so the square-accum
            # carries the +1 of s = 1 + 2 r IP + ||P||^2 through the reduce.
            P_ring = []
            for i in range(4):
                pring_tile = cpool.tile([128, 5], f32, tag=f"Pring{i}")
                P_ring.append(pring_tile)
                nc.vector.memset(pring_tile[:, 4:5], float(np.sqrt(1.0 / 128.0)))

            from contextlib import nullcontext

            rep_ctx = tc.For_i(0, loop_reps, 1) if loop_reps > 1 else nullcontext()
            with rep_ctx:
                u1, u2 = v0_sb, v0_sb
                r1 = ones_ap
                grA, grB = gp_ap, gp_ap
                rp_prev = None
                for c in range(n_chunks):
                    t0 = c * PCH
                    x_sb = xpool.tile([128, H], f32, tag="x")
                    nc.sync.dma_start(x_sb, xb[t0 : t0 + PCH, :])

                    xT_sb = xpool.tile([128, H], f32, tag="xT")
                    pe_touch(x_sb[:, 0:128])
                    for hi in range(8):
                        tps = ps_t.tile([128, 128], f32, tag="tps")
                        nc.tensor.transpose(
                            tps, x_sb[:, hi * 128 : (hi + 1) * 128], ident[:]
                        )
                        nc.scalar.copy(xT_sb[:, hi * 128 : (hi + 1) * 128], tps[:])

                    m_ps = ps_m.tile([128, 256], f32, tag="m")
                    for hi in range(8):
                        nc.tensor.matmul(
                            m_ps[:],
                            xT_sb[:, hi * 128 : (hi + 1) * 128],
                            bm_sb[:, hi * 256 : (hi + 1) * 256],
                            start=(hi == 0),
                            stop=(hi == 7),
                        )

                    mag = wpool.tile([128, 256], f32, tag="mag")
                    nc.scalar.activation(mag[:], m_ps[:], Act.Abs)
                    mx8 = wpool.tile([128, 8], f32, tag="mx8")
                    nc.vector.max(mx8[:], mag[:])
                    inj = wpool.tile([128, 256], f32, tag="inj")
                    nc.vector.scalar_tensor_tensor(
                        out=inj[:], in0=mag[:], scalar=mx8[:, 7:8], in1=m_ps[:],
                        op0=Alu.is_ge, op1=Alu.mult,
                    )

                    injT = wpool.tile([128, 512], f32, tag="injT")
                    nc.gpsimd.memset(injT[:, 256:512], 0.0)
                    for q in range(2):
                        tps = ps_t.tile([128, 128], f32, tag="tps")
                        nc.tensor.transpose(
                            tps, inj[:, q * 128 : (q + 1) * 128], ident[:]
                        )
                        nc.scalar.copy(injT[:, q * 128 : (q + 1) * 128], tps[:])

                    z_sb = wpool.tile([128, 256], f32, tag="z")

                    for k in range(PCH // 2):
                        a = 2 * k
                        inj_a = injT[:, a : 512 : 128]
                        inj_b = injT[:, a + 1 : 512 : 128]
                        Pa = P_ring[a % 4]
                        Pb = P_ring[(a + 1) % 4]
                        sq = spool.tile([128, 4], f32, tag="sq")

                        # DVE: everything feeding the reduce (single PE wait).
                        # gr for this pair is computed here (DVE waits on the
                        # previous pair's r anyway) instead of trailing on ACT
                        # where it would delay this pair's DVE start.
                        if gr_eng == "dve" and rp_prev is not None:
                            grp = spool.tile([128, 2], f32, tag="grp")
                            nc.vector.tensor_scalar_mul(grp[:], rp_prev[:], gp_ap)
                            grA, grB = grp[:, 0:1], grp[:, 1:2]
                        nc.vector.scalar_tensor_tensor(
                            out=Pa[:, 0:4], in0=u2[:], scalar=grA, in1=inj_a,
                            op0=Alu.mult, op1=Alu.add,
                        )
                        d0a = spool.tile([128, 4], f32, tag="d0a")
                        nc.vector.scalar_tensor_tensor(
                            out=d0a[:], in0=u1[:], scalar=2.0, in1=Pa[:, 0:4],
                            op0=Alu.mult, op1=Alu.mult, accum_out=sq[:, 0:1],
                        )
                        d1a = spool.tile([128, 5], f32, tag="d1a")
                        nc.vector.scalar_tensor_tensor(
                            out=d1a[:], in0=Pa[:], scalar=1.0, in1=Pa[:],
                            op0=Alu.mult, op1=Alu.mult, accum_out=sq[:, 1:2],
                        )
                        u_a = spool.tile([128, 4], f32, tag="u")
                        nc.vector.scalar_tensor_tensor(
                            out=u_a[:], in0=u1[:], scalar=r1, in1=Pa[:, 0:4],
                            op0=Alu.mult, op1=Alu.add,
                        )
                        # P_b on pool (no r dependence)
                        p_b_e = getattr(nc, P_ENG[p_b_eng])
                        p_b_e.scalar_tensor_tensor(
                            out=Pb[:, 0:4], in0=u1[:], scalar=grB, in1=inj_b,
                            op0=Alu.mult, op1=Alu.add,
                        )
                        d0b = spool.tile([128, 4], f32, tag="d0b")
                        nc.vector.scalar_tensor_tensor(
                            out=d0b[:], in0=u_a[:], scalar=2.0, in1=Pb[:, 0:4],
                            op0=Alu.mult, op1=Alu.mult, accum_out=sq[:, 2:3],
                        )
                        d1b = spool.tile([128, 5], f32, tag="d1b")
                        nc.vector.scalar_tensor_tensor(
                            out=d1b[:], in0=Pb[:], scalar=1.0, in1=Pb[:],
                            op0=Alu.mult, op1=Alu.mult, accum_out=sq[:, 3:4],
                        )

                        sq4r = ps_r.tile([128, 4], f32, tag="ps_b")
                        nc.tensor.matmul(
                            sq4r[:], ones_mat[:], sq[:], start=True, stop=True
                        )

                        pp = spool.tile([128, 2], f32, tag="pp")
                        nc.scalar.copy(pp[:], sq4r[:, 1:4:2])
                        rp = spool.tile([128, 2], f32, tag="rp")
                        if r_mode == "abs_rsqrt":
                            nc.scalar.activation(
                                rp[:, 0:1], sq4r[:, 0:1], Act.Abs_reciprocal_sqrt,
                                scale=r1, bias=pp[:, 0:1],
                            )
                            nc.scalar.activation(
                                rp[:, 1:2], sq4r[:, 2:3], Act.Abs_reciprocal_sqrt,
                                scale=rp[:, 0:1], bias=pp[:, 1:2],
                            )
                        else:
                            n_a = spool.tile([128, 2], f32, tag="n")
                            nc.scalar.activation(
                                n_a[:, 0:1], sq4r[:, 0:1], Act.Sqrt,
                                scale=r1, bias=pp[:, 0:1],
                            )
                            nc.vector.reciprocal(rp[:, 0:1], n_a[:, 0:1])
                            nc.scalar.activation(
                                n_a[:, 1:2], sq4r[:, 2:3], Act.Sqrt,
                                scale=rp[:, 0:1], bias=pp[:, 1:2],
                            )
                            nc.vector.reciprocal(rp[:, 1:2], n_a[:, 1:2])

                        u_b = spool.tile([128, 4], f32, tag="u")
                        u_b_e = getattr(nc, P_ENG[u_b_eng])
                        u_b_e.scalar_tensor_tensor(
                            out=u_b[:], in0=u_a[:], scalar=rp[:, 0:1],
                            in1=Pb[:, 0:4], op0=Alu.mult, op1=Alu.add,
                        )
                        if gr_eng == "act":
                            grp = spool.tile([128, 2], f32, tag="grp")
                            nc.scalar.activation(
                                grp[:], rp[:], Act.Copy, scale=gp_ap
                            )
                            grA, grB = grp[:, 0:1], grp[:, 1:2]
                        elif gr_eng == "pool":
                            grp = spool.tile([128, 2], f32, tag="grp")
                            nc.gpsimd.tensor_scalar_mul(grp[:], rp[:], gp_ap)
                            grA, grB = grp[:, 0:1], grp[:, 1:2]
                        if z_eng == "act":
                            nc.scalar.activation(
                                z_sb[:, a : 256 : 128], u_a[:, 0:2], Act.Copy,
                                scale=rp[:, 0:1],
                            )
                            nc.scalar.activation(
                                z_sb[:, a + 1 : 256 : 128], u_b[:, 0:2], Act.Copy,
                                scale=rp[:, 1:2],
                            )
                        else:
                            z_e = getattr(nc, P_ENG[z_eng])
                            z_e.tensor_scalar_mul(
                                z_sb[:, a : 256 : 128], u_a[:, 0:2], rp[:, 0:1]
                            )
                            z_e.tensor_scalar_mul(
                                z_sb[:, a + 1 : 256 : 128], u_b[:, 0:2], rp[:, 1:2]
                            )

                        u2, u1 = u_a, u_b
                        r1 = rp[:, 1:2]
                        rp_prev = rp

                    y_sb = xpool.tile([128, H], f32, tag="y")
                    for hi in range(8):
                        yT_ps = ps_y.tile([128, 128], f32, tag="yT")
                        nc.tensor.matmul(
                            yT_ps[:], by_sb[:, (hi * 2 + 0) * 128 : (hi * 2 + 1) * 128],
                            z_sb[:, 0:128], start=True, stop=False,
                        )
                        nc.tensor.matmul(
                            yT_ps[:], by_sb[:, (hi * 2 + 1) * 128 : (hi * 2 + 2) * 128],
                            z_sb[:, 128:256], start=False, stop=False,
                        )
                        nc.tensor.matmul(
                            yT_ps[:], ident[:], xT_sb[:, hi * 128 : (hi + 1) * 128],
                            start=False, stop=True,
                        )
                        yT_sb = wpool.tile([128, 128], f32, tag="yTs")
                        nc.scalar.copy(yT_sb[:], yT_ps[:])
                        y_ps = ps_y.tile([128, 128], f32, tag="yT")
                        nc.tensor.transpose(y_ps, yT_sb[:], ident[:])
                        nc.scalar.copy(y_sb[:, hi * 128 : (hi + 1) * 128], y_ps[:])
                    nc.sync.dma_start(yb[t0 : t0 + PCH, :], y_sb[:])

    nc.compile()
    return nc


def _host_pack(inputs):
    """Fold all small parameters host-side; returns per-core constant arrays."""
    basis = np.asarray(inputs["basis"], np.float32)
    alpha = float(np.asarray(inputs["alpha"]))
    w_r = np.asarray(inputs["w_r"], np.float32)
    bg = _sigmoid(np.asarray(inputs["breadth_gate"], np.float32))

    g = _sigmoid(w_r)
    assert np.all(g[:MEM] == g[0]), "vector w_r gate not supported by fast path"
    gp = float(g[0]) / GAMMA

    Wm = (basis[:, :MEM] * (BETA / GAMMA)).astype(np.float32)  # [H, 256]
    Wy = (basis[:, :MEM] * (alpha * bg[None, :MEM])).astype(np.float32)

    # basis_m blocks: block hi = Wm[hi*128:(hi+1)*128, :]  -> cols [hi*256, ...)
    basis_m = np.concatenate(
        [Wm[hi * 128 : (hi + 1) * 128, :] for hi in range(8)], axis=1
    ).astype(np.float32)  # [128, 2048]
    WyT = np.ascontiguousarray(Wy.T)  # [256, 1024]
    blocks = []
    for hi in range(8):
        for q in range(2):
            blocks.append(WyT[q * 128 : (q + 1) * 128, hi * 128 : (hi + 1) * 128])
    basis_y = np.concatenate(blocks, axis=1).astype(np.float32)  # [128, 2048]

    t0c = (
        np.asarray(inputs["tape_init_re"], np.float32)
        + 1j * np.asarray(inputs["tape_init_im"], np.float32)
    )[:MEM].astype(np.complex64)
    nrm = np.float32(np.sqrt(max(float((np.abs(t0c) ** 2).sum(dtype=np.float32)), 1e-16)))
    v0c = (t0c / nrm).astype(np.complex64)
    v0 = np.stack(
        [v0c.real[:128], v0c.real[128:], v0c.imag[:128], v0c.imag[128:]], axis=1
    ).astype(np.float32)  # [128, 4]

    scal = np.empty((128, 3), np.float32)
    scal[:, 0] = 1.0
    scal[:, 1] = gp
    scal[:, 2] = 1.0
    return basis_m, basis_y, v0, scal


def _fast_path_ok(inputs):
    z = lambda k: np.all(np.asarray(inputs[k]) == 0)
    g = _sigmoid(np.asarray(inputs["w_r"], np.float32))
    return (
        z("torque_rotation")
        and z("epsilon_scale")
        and z("epsilon_diag")
        and z("pred_scale")
        and z("pred_diag")
        and bool(np.all(g[:MEM] == g[0]))
    )


def _numpy_fallback(inputs):
    """General-case reference implementation (host). Only used if the inputs
    violate the fast-path structure (never the case for this problem's
    generator); keeps kernel() total."""
    import jax

    with jax.default_device(jax.devices("cpu")[0]):
        import jax.numpy as jnp
        from jax import lax

        x = jnp.asarray(inputs["x"])
        basis = jnp.asarray(inputs["basis"])
        active = jnp.arange(S) < MEM
        amf = active.astype(jnp.float32)
        eta = jax.nn.softplus(jnp.asarray(inputs["eta_raw"]))
        eps = (jnp.asarray(inputs["epsilon_factor"]) * jnp.asarray(inputs["epsilon_scale"])) @ jnp.asarray(
            inputs["epsilon_factor"]).T + jnp.diag(jnp.asarray(inputs["epsilon_diag"]))
        wp = (jnp.asarray(inputs["pred_factor"]) * jnp.asarray(inputs["pred_scale"])) @ jnp.asarray(
            inputs["pred_factor"]).T + jnp.diag(jnp.asarray(inputs["pred_diag"]))
        eps_c = eps.astype(jnp.complex64)
        wp_c = wp.astype(jnp.complex64)
        rot = jnp.exp(1j * jnp.asarray(inputs["torque_rotation"]).astype(jnp.complex64))
        wr_gate = jax.nn.sigmoid(jnp.asarray(inputs["w_r"]))
        bg = jax.nn.sigmoid(jnp.asarray(inputs["breadth_gate"]))
        alpha = jnp.asarray(inputs["alpha"])

        def renorm(tape):
            masked = tape * amf
            nrm = jnp.sqrt(jnp.maximum((jnp.abs(masked) ** 2).sum(-1, keepdims=True), 1e-16))
            return masked / nrm

        tape0 = (jnp.asarray(inputs["tape_init_re"]) + 1j * jnp.asarray(inputs["tape_init_im"])) * amf
        tape0 = renorm(jnp.broadcast_to(tape0, (B, S)))

        def step(carry, x_t):
            tape, prev = carry
            m = jnp.einsum("hs,bh->bs", basis, x_t)
            mag = jnp.abs(m) * amf
            kth = lax.top_k(mag, TOPK)[0][:, -1:]
            injv = jnp.where((mag >= kth) & active, m, 0.0).astype(jnp.complex64)
            rotated = tape * rot
            drive = jnp.einsum("st,bt->bs", eps_c, rotated)
            pred = jnp.einsum("st,bt->bs", wp_c, rotated)
            new = (GAMMA * rotated + eta * drive + BETA * injv + PTS * 1j * pred + wr_gate * prev)
            new = renorm(new)
            y = x_t + alpha * jnp.einsum("hs,bs->bh", basis, bg * new.real)
            return (new, tape), y

        (_, _), ys = lax.scan(step, (tape0, tape0), jnp.swapaxes(x, 0, 1))
        return np.asarray(jnp.swapaxes(ys, 0, 1))


USE_V2 = False
BEST_KW = {"use_rsqrt": True}


def _timing_build(n_chunks: int, loop_reps: int = 1):
    """Builder used by kernel() and test.py's repetition timer."""
    build = _build_program_v2 if USE_V2 else _build_program
    return build(n_chunks, loop_reps=loop_reps, **BEST_KW)


def kernel(n_chunks: int = T // PCH, _want_trace: bool = False, **inputs) -> np.ndarray:
    from concourse.bass_utils import run_bass_kernel_spmd

    x = np.ascontiguousarray(np.asarray(inputs["x"], np.float32))
    assert x.shape == (B, T, H)

    if not _fast_path_ok(inputs):
        return _numpy_fallback(inputs)

    basis_m, basis_y, v0, scal = _host_pack(inputs)

    key = (n_chunks, USE_V2, tuple(sorted(BEST_KW.items())))
    if key not in _program_cache:
        _program_cache[key] = _timing_build(n_chunks)
    nc = _program_cache[key]

    Tq = n_chunks * PCH
    core_ids = list(range(B))
    in_maps = [
        {
            "xb": np.ascontiguousarray(x[b, :Tq]),
            "basis_m": basis_m,
            "basis_y": basis_y,
            "v0": v0,
            "scal": scal,
        }
        for b in core_ids
    ]
    res = run_bass_kernel_spmd(nc, in_maps, core_ids, trace=_want_trace)
    out = np.empty((B, Tq, H), np.float32)
    for b in core_ids:
        out[b] = res.results[b]["yb"]
    if _want_trace:
        kernel._last_results = res
    return out



# revision 22
# speedup vs baseline: 1.4150x; 1.4150x over previous
"""Trainium2 Bass kernel for nn_MemoryEngineLayer (scatter_memory).

Contract: kernel(**inputs) takes FULL unsharded inputs (as produced by
setup_inputs()) and returns the FULL [B, T, H] output. Internally the batch
dim (B=8) is sharded across 8 NeuronCores (pure data parallelism); each core
runs the T=4096 recurrent scan for its own sequence.

Math (validated against the jax reference, rel err ~1e-8):
  m'_t   = x_t @ (beta/gamma * basis[:, :256])           # [256], top-8 inject
  inj'_t = where(|m'_t| >= kth8(|m'_t|), m'_t, 0)
  u_t    = r_{t-1} u_{t-1} + (g/gamma) r_{t-2} u_{t-2} + inj'_t   (complex 256)
  s_t    = 1 + 2 r_{t-1} <u_{t-1}, P_t> + ||P_t||^2,  P_t = g' r_{t-2} u_{t-2} + inj'_t
  r_t    = 1/sqrt(s_t + 1e-16)        # scale-invariant renorm
  y_t    = x_t + Re(r_t u_t) @ (alpha * bg * basis[:, :256]).T

On-chip layout: complex state as [128 partitions, 4 cols] = (re_lo, re_hi,
im_lo, im_hi) with slot s = q*128 + p. The per-step norm uses the identity
||r u_{t-1}|| = 1, keeping the partition_all_reduce off the r-dependency
chain.
"""

import numpy as np

H, MEM, S = 1024, 256, 272
B, T = 8, 4096
TOPK = 8
GAMMA, BETA, PTS = 0.92, 0.08, 0.4
PCH = 128  # timesteps per chunk

_program_cache = {}


def _sigmoid(v):
    return 1.0 / (1.0 + np.exp(-v.astype(np.float64)))


def _build_program(n_chunks: int, loop_reps: int = 1, pp_on_act: bool = False, fused_n: bool = True, z_on_dve: bool = False, gr_on_dve: bool = False, t_no_allred: bool = False, t_no_chain: bool = False, pe_allred: bool = True, use_rsqrt: bool = False, scan_bufs: int = 4, psr_bufs: int = 2, pring_n: int = 4):
    import concourse.bass as bass
    import concourse.bacc as bacc
    import concourse.mybir as mybir
    from concourse.tile import TileContext
    from concourse.masks import make_identity
    from concourse import bass_isa

    f32 = mybir.dt.float32
    Alu = mybir.AluOpType
    Act = mybir.ActivationFunctionType
    Tq = n_chunks * PCH

    from concourse._compat import get_trn_type
    nc = bacc.Bacc(get_trn_type() or "TRN2", target_bir_lowering=False, debug=False)
    xb = nc.declare_dram_parameter("xb", [Tq, H], f32, isOutput=False)
    basis_m = nc.declare_dram_parameter("basis_m", [128, 8 * 256], f32, isOutput=False)
    basis_y = nc.declare_dram_parameter("basis_y", [128, 16 * 128], f32, isOutput=False)
    v0d = nc.declare_dram_parameter("v0", [128, 4], f32, isOutput=False)
    scal = nc.declare_dram_parameter("scal", [128, 3], f32, isOutput=False)  # [ones, gp, one]
    yb = nc.declare_dram_parameter("yb", [Tq, H], f32, isOutput=True)

    with TileContext(nc) as tc:
        with (
            tc.tile_pool(name="const", bufs=1) as cpool,
            tc.tile_pool(name="xio", bufs=4) as xpool,
            tc.tile_pool(name="work", bufs=2) as wpool,
            tc.tile_pool(name="scan", bufs=scan_bufs) as spool,
            tc.tile_pool(name="ps_t", bufs=2, space="PSUM") as ps_t,
            tc.tile_pool(name="ps_m", bufs=1, space="PSUM") as ps_m,
            tc.tile_pool(name="ps_y", bufs=2, space="PSUM") as ps_y,
            tc.tile_pool(name="ps_r", bufs=psr_bufs, space="PSUM") as ps_r,
        ):
            # ---- constants ----
            bm_sb = cpool.tile([128, 8 * 256], f32, tag="bm")
            nc.sync.dma_start(bm_sb, basis_m[:])
            by_sb = cpool.tile([128, 16 * 128], f32, tag="by")
            nc.sync.dma_start(by_sb, basis_y[:])
            v0_sb = cpool.tile([128, 4], f32, tag="v0")
            nc.sync.dma_start(v0_sb, v0d[:])
            sc_sb = cpool.tile([128, 3], f32, tag="sc")
            nc.sync.dma_start(sc_sb, scal[:])
            ident = cpool.tile([128, 128], f32, tag="ident")
            make_identity(nc, ident[:])
            ones_row = cpool.tile([1, 128], f32, tag="ones_row")
            nc.vector.memset(ones_row[:], 1.0)
            ones_mat = cpool.tile([128, 128], f32, tag="ones_mat")
            nc.vector.memset(ones_mat[:], 1.0)

            ones_ap = sc_sb[:, 0:1]
            gp_ap = sc_sb[:, 1:2]
            one_ap = sc_sb[:, 2:3]

            # Warm-ups: walrus allows at most ONE sync wait per PE matmul, so
            # make each engine observe the constant tiles via single-dep ops
            # before any real consumer needs them together.
            warm_acc = ps_m.tile([128, 1], f32, tag="warm")

            def pe_touch(sb_slice):
                # tiny N=1 matmul whose ONLY fresh dep is sb_slice; writes to
                # the shared warm PSUM tile (same-engine WAW, no semaphore)
                nc.tensor.matmul(
                    warm_acc[:], sb_slice, ident[:, 0:1], start=True, stop=True
                )

            pe_touch(ident[:, 0:128])  # waits Pool (make_identity) only
            pe_touch(bm_sb[:, 0:128])  # waits bm DMA only
            pe_touch(by_sb[:, 0:128])  # waits by DMA only
            wsc = spool.tile([128, 4], f32, tag="wsc")
            nc.vector.tensor_copy(wsc[:], v0_sb[:])  # DVE observes v0 DMA
            wsc2 = spool.tile([128, 3], f32, tag="wsc2")
            nc.vector.tensor_copy(wsc2[:], sc_sb[:])  # DVE observes scal DMA
            wsc3 = spool.tile([128, 3], f32, tag="wsc3")
            nc.scalar.copy(wsc3[:], sc_sb[:])  # ACT observes scal DMA

            # scan state carried across chunks (python vars reference tiles)
            u1, u2 = v0_sb, v0_sb          # u_{t-1}, u_{t-2}
            r1 = ones_ap                    # r_{t-1} [128,1]
            # FIFO of gp*r values: step t pops gp*r_{t-2}, pushes gp*r_t
            gr_fifo = [gp_ap, gp_ap]        # gp*r_{-2}, gp*r_{-1}

            # P ring: 4 persistent [128,5] tiles; cols 0:4 hold P_t, col 4 is
            # the constant sqrt(1/128) so ACT Square+accum yields
            # sum(P^2) + 1/128 per partition -> +1 total after the
            # partition_all_reduce (the "1" of s = 1 + 2 r IP + ||P||^2).
            P_ring = []
            for i in range(pring_n):
                pring_tile = cpool.tile([128, 5], f32, tag=f"Pring{i}")
                P_ring.append(pring_tile)
                nc.vector.memset(pring_tile[:, 4:5], float(np.sqrt(1.0 / 128.0)))

            from contextlib import nullcontext
            rep_ctx = tc.For_i(0, loop_reps, 1) if loop_reps > 1 else nullcontext()
            with rep_ctx:
              # (re)bind scan state at body start so each repetition restarts
              u1, u2 = v0_sb, v0_sb
              r1 = ones_ap
              gr_fifo = [gp_ap, gp_ap]
              for c in range(n_chunks):
                  t0 = c * PCH
                  x_sb = xpool.tile([128, H], f32, tag="x")
                  nc.sync.dma_start(x_sb, xb[t0 : t0 + PCH, :])

                  # transpose x chunk -> xT [h, t] blocks
                  xT_sb = xpool.tile([128, H], f32, tag="xT")
                  # single-dep touch so the real transposes don't combine the
                  # x-DMA wait with a PSUM-slot-release wait
                  pe_touch(x_sb[:, 0:128])
                  for hi in range(8):
                      tps = ps_t.tile([128, 128], f32, tag="tps")
                      nc.tensor.transpose(tps, x_sb[:, hi * 128 : (hi + 1) * 128], ident[:])
                      nc.scalar.copy(xT_sb[:, hi * 128 : (hi + 1) * 128], tps[:])

                  # m' = x @ Wm  -> psum [t=128, s=256]
                  m_ps = ps_m.tile([128, 256], f32, tag="m")
                  for hi in range(8):
                      nc.tensor.matmul(
                          m_ps[:],
                          xT_sb[:, hi * 128 : (hi + 1) * 128],
                          bm_sb[:, hi * 256 : (hi + 1) * 256],
                          start=(hi == 0),
                          stop=(hi == 7),
                      )

                  # top-8 threshold + injection
                  mag = wpool.tile([128, 256], f32, tag="mag")
                  nc.scalar.activation(mag[:], m_ps[:], Act.Abs)
                  mx8 = wpool.tile([128, 8], f32, tag="mx8")
                  nc.vector.max(mx8[:], mag[:])
                  inj = wpool.tile([128, 256], f32, tag="inj")
                  nc.vector.scalar_tensor_tensor(
                      out=inj[:], in0=mag[:], scalar=mx8[:, 7:8], in1=m_ps[:],
                      op0=Alu.is_ge, op1=Alu.mult,
                  )

                  # transpose inj -> injT [slot, t] with zero upper half (imag)
                  injT = wpool.tile([128, 512], f32, tag="injT")
                  nc.gpsimd.memset(injT[:, 256:512], 0.0)
                  for q in range(2):
                      tps = ps_t.tile([128, 128], f32, tag="tps")
                      nc.tensor.transpose(tps, inj[:, q * 128 : (q + 1) * 128], ident[:])
                      nc.scalar.copy(injT[:, q * 128 : (q + 1) * 128], tps[:])

                  z_sb = wpool.tile([128, 256], f32, tag="z")

                  # ---- the sequential scan: 128 steps ----
                  for t in range(PCH):
                      inj4 = injT[:, t : 512 : 128]  # (re_lo, re_hi, 0, 0)
                      gr2 = gr_fifo.pop(0) if not t_no_chain else gp_ap
                      P = P_ring[t % pring_n]
                      nc.vector.scalar_tensor_tensor(
                          out=P[:, 0:4], in0=u2[:], scalar=gr2, in1=inj4,
                          op0=Alu.mult, op1=Alu.add,
                      )
                      u = spool.tile([128, 4], f32, tag="u")
                      nc.vector.scalar_tensor_tensor(
                          out=u[:], in0=u1[:], scalar=(ones_ap if t_no_chain else r1), in1=P[:, 0:4],
                          op0=Alu.mult, op1=Alu.add,
                      )
                      # sq2 col0 = sum(2*u1*P) (DVE), col1 = sum(P^2)+1/128 (ACT)
                      # (tensor_tensor_reduce crashes the device on this runtime;
                      # stt accum_out / ACT Square accum_out are the variants
                      # that work)
                      sq2 = spool.tile([128, 2], f32, tag="sq2")
                      d0 = spool.tile([128, 4], f32, tag="d0")
                      nc.vector.scalar_tensor_tensor(
                          out=d0[:], in0=u1[:], scalar=2.0, in1=P[:, 0:4],
                          op0=Alu.mult, op1=Alu.mult, accum_out=sq2[:, 0:1],
                      )
                      d1 = spool.tile([128, 5], f32, tag="d1")
                      if pp_on_act:
                          nc.scalar.activation(
                              d1[:], P[:], Act.Square, accum_out=sq2[:, 1:2]
                          )
                      else:
                          nc.vector.scalar_tensor_tensor(
                              out=d1[:], in0=P[:], scalar=1.0, in1=P[:],
                              op0=Alu.mult, op1=Alu.mult, accum_out=sq2[:, 1:2],
                          )
                      if pe_allred and not t_no_allred:
                          # one all-ones matmul = partition collapse AND
                          # broadcast (the Pool partition_all_reduce costs
                          # ~1us/step on HW)
                          sq2r = ps_r.tile([128, 2], f32, tag="ps_b")
                          nc.tensor.matmul(
                              sq2r[:], ones_mat[:], sq2[:], start=True, stop=True
                          )
                          # PP+1 must live in SBUF (stt can't read two PSUM
                          # operands; ACT bias must be SBUF)
                          pp1_sb = spool.tile([128, 1], f32, tag="pp1")
                          nc.scalar.copy(pp1_sb[:], sq2r[:, 1:2])
                      else:
                          sq2r = spool.tile([128, 2], f32, tag="sq2r")
                          if t_no_allred:  # timing-only: wrong math
                              nc.vector.tensor_copy(sq2r[:], sq2[:])
                          else:
                              nc.gpsimd.partition_all_reduce(
                                  sq2r[:], sq2[:], 128, bass_isa.ReduceOp.add
                              )
                      rsqrt_path = use_rsqrt and pe_allred and not t_no_chain
                      n_t = None
                      if not rsqrt_path:
                          n_t = spool.tile([128, 1], f32, tag="n")
                      if t_no_chain:  # timing-only: wrong math, breaks r recurrence
                          s_t = spool.tile([128, 1], f32, tag="s")
                          nc.vector.scalar_tensor_tensor(
                              out=s_t[:], in0=sq2r[:, 0:1], scalar=ones_ap, in1=sq2r[:, 1:2],
                              op0=Alu.mult, op1=Alu.add,
                          )
                          nc.scalar.activation(n_t[:], s_t[:], Act.Sqrt)
                      elif rsqrt_path:
                          # r = rsqrt(2*IP_tot*r1 + (PP_tot+1)) in one ACT op —
                          # removes the DVE reciprocal from the chain entirely
                          pass
                      elif fused_n and pe_allred:
                          # n = sqrt(2*IP_tot*r1 + (PP_tot+1)) in one ACT op,
                          # reading 2IP from PSUM and PP+1 from SBUF
                          nc.scalar.activation(
                              n_t[:], sq2r[:, 0:1], Act.Sqrt, scale=r1, bias=pp1_sb[:]
                          )
                      elif fused_n:
                          # n = sqrt(2*IP*r1 + (||P||^2 + 1)) in one ACT op
                          nc.scalar.activation(
                              n_t[:], sq2r[:, 0:1], Act.Sqrt, scale=r1, bias=sq2r[:, 1:2]
                          )
                      else:
                          s_t = spool.tile([128, 1], f32, tag="s")
                          nc.vector.scalar_tensor_tensor(
                              out=s_t[:], in0=sq2r[:, 0:1], scalar=r1, in1=sq2r[:, 1:2],
                              op0=Alu.mult, op1=Alu.add,
                          )
                          nc.scalar.activation(n_t[:], s_t[:], Act.Sqrt)
                      r_t = spool.tile([128, 1], f32, tag="r")
                      if rsqrt_path:
                          # Abs_reciprocal_sqrt == rsqrt for s>0 (4e-5 rel err
                          # measured on HW); one ACT op replaces Sqrt + DVE
                          # reciprocal, removing an edge from the r-chain
                          nc.scalar.activation(
                              r_t[:], sq2r[:, 0:1], Act.Abs_reciprocal_sqrt,
                              scale=r1, bias=pp1_sb[:],
                          )
                      else:
                          nc.vector.reciprocal(r_t[:], n_t[:])
                      gr_t = spool.tile([128, 1], f32, tag="gr")
                      if gr_on_dve:
                          nc.vector.tensor_scalar_mul(gr_t[:], r_t[:], gp_ap)
                      else:
                          nc.scalar.activation(gr_t[:], r_t[:], Act.Copy, scale=gp_ap)
                      # z_t = Re(r_t * u_t)
                      if z_on_dve:
                          nc.vector.tensor_scalar_mul(
                              z_sb[:, t : 256 : 128], u[:, 0:2], r_t[:]
                          )
                      else:
                          nc.scalar.activation(
                              z_sb[:, t : 256 : 128], u[:, 0:2], Act.Copy, scale=r_t[:]
                          )
                      u2, u1 = u1, u
                      gr_fifo.append(gr_t[:])
                      r1 = r_t[:]

                  # ---- y^T = Wy^T @ z + x^T ; transpose back; store ----
                  y_sb = xpool.tile([128, H], f32, tag="y")
                  for hi in range(8):
                      yT_ps = ps_y.tile([128, 128], f32, tag="yT")
                      nc.tensor.matmul(
                          yT_ps[:], by_sb[:, (hi * 2 + 0) * 128 : (hi * 2 + 1) * 128],
                          z_sb[:, 0:128], start=True, stop=False,
                      )
                      nc.tensor.matmul(
                          yT_ps[:], by_sb[:, (hi * 2 + 1) * 128 : (hi * 2 + 2) * 128],
                          z_sb[:, 128:256], start=False, stop=False,
                      )
                      nc.tensor.matmul(
                          yT_ps[:], ident[:], xT_sb[:, hi * 128 : (hi + 1) * 128],
                          start=False, stop=True,
                      )
                      yT_sb = wpool.tile([128, 128], f32, tag="yTs")
                      nc.scalar.copy(yT_sb[:], yT_ps[:])
                      y_ps = ps_y.tile([128, 128], f32, tag="yT")
                      nc.tensor.transpose(y_ps, yT_sb[:], ident[:])
                      nc.scalar.copy(y_sb[:, hi * 128 : (hi + 1) * 128], y_ps[:])
                  nc.sync.dma_start(yb[t0 : t0 + PCH, :], y_sb[:])

    nc.compile()
    return nc


def _build_program_p(n_chunks: int, loop_reps: int = 1, scan_bufs: int = 4,
                     pring_n: int = 4):
    """Software-pipelined v1+rsqrt: chunk c+1's prep (x DMA, transposes,
    m-matmul, top-8 inject, injT) is emitted in pieces after scan steps
    64.. of chunk c, and chunk c-1's y-block is emitted in pieces after
    steps 4..52 — so the per-chunk boundary bubble (~8-11us of serialized
    PE/ACT work) is absorbed into the scan's engine idle gaps."""
    import concourse.bass as bass
    import concourse.bacc as bacc
    import concourse.mybir as mybir
    from concourse.tile import TileContext
    from concourse.masks import make_identity

    f32 = mybir.dt.float32
    Alu = mybir.AluOpType
    Act = mybir.ActivationFunctionType
    Tq = n_chunks * PCH

    from concourse._compat import get_trn_type

    nc = bacc.Bacc(get_trn_type() or "TRN2", target_bir_lowering=False, debug=False)
    xb = nc.declare_dram_parameter("xb", [Tq, H], f32, isOutput=False)
    basis_m = nc.declare_dram_parameter("basis_m", [128, 8 * 256], f32, isOutput=False)
    basis_y = nc.declare_dram_parameter("basis_y", [128, 16 * 128], f32, isOutput=False)
    v0d = nc.declare_dram_parameter("v0", [128, 4], f32, isOutput=False)
    scal = nc.declare_dram_parameter("scal", [128, 3], f32, isOutput=False)
    yb = nc.declare_dram_parameter("yb", [Tq, H], f32, isOutput=True)

    with TileContext(nc) as tc:
        with (
            tc.tile_pool(name="const", bufs=1) as cpool,
            tc.tile_pool(name="xio", bufs=4) as xpool,
            tc.tile_pool(name="work", bufs=2) as wpool,
            tc.tile_pool(name="scan", bufs=scan_bufs) as spool,
            tc.tile_pool(name="ps_t", bufs=2, space="PSUM") as ps_t,
            tc.tile_pool(name="ps_m", bufs=1, space="PSUM") as ps_m,
            tc.tile_pool(name="ps_y", bufs=2, space="PSUM") as ps_y,
            tc.tile_pool(name="ps_r", bufs=2, space="PSUM") as ps_r,
        ):
            bm_sb = cpool.tile([128, 8 * 256], f32, tag="bm")
            nc.sync.dma_start(bm_sb, basis_m[:])
            by_sb = cpool.tile([128, 16 * 128], f32, tag="by")
            nc.sync.dma_start(by_sb, basis_y[:])
            v0_sb = cpool.tile([128, 4], f32, tag="v0")
            nc.sync.dma_start(v0_sb, v0d[:])
            sc_sb = cpool.tile([128, 3], f32, tag="sc")
            nc.sync.dma_start(sc_sb, scal[:])
            ident = cpool.tile([128, 128], f32, tag="ident")
            make_identity(nc, ident[:])
            ones_mat = cpool.tile([128, 128], f32, tag="ones_mat")
            nc.vector.memset(ones_mat[:], 1.0)

            ones_ap = sc_sb[:, 0:1]
            gp_ap = sc_sb[:, 1:2]

            warm_acc = ps_m.tile([128, 1], f32, tag="warm")

            def pe_touch(sb_slice):
                nc.tensor.matmul(
                    warm_acc[:], sb_slice, ident[:, 0:1], start=True, stop=True
                )

            pe_touch(ident[:, 0:128])
            pe_touch(bm_sb[:, 0:128])
            pe_touch(by_sb[:, 0:128])
            wsc = spool.tile([128, 4], f32, tag="wsc")
            nc.vector.tensor_copy(wsc[:], v0_sb[:])
            wsc2 = spool.tile([128, 3], f32, tag="wsc2")
            nc.vector.tensor_copy(wsc2[:], sc_sb[:])
            wsc3 = spool.tile([128, 3], f32, tag="wsc3")
            nc.scalar.copy(wsc3[:], sc_sb[:])

            P_ring = []
            for i in range(pring_n):
                pring_tile = cpool.tile([128, 5], f32, tag=f"Pring{i}")
                P_ring.append(pring_tile)
                nc.vector.memset(pring_tile[:, 4:5], float(np.sqrt(1.0 / 128.0)))

            def prep_pieces(c):
                """Returns (st, pieces): st is filled in as pieces run."""
                t0 = c * PCH
                st = {}
                pieces = []

                def p_dma():
                    st["x"] = xpool.tile([128, H], f32, tag="x", name=f"xch{c}")
                    nc.sync.dma_start(st["x"], xb[t0 : t0 + PCH, :])
                    st["xT"] = xpool.tile([128, H], f32, tag="xT", name=f"xTch{c}")
                    pe_touch(st["x"][:, 0:128])

                pieces.append(p_dma)
                for hi in range(8):
                    def p_tr(hi=hi):
                        tps = ps_t.tile([128, 128], f32, tag="tps")
                        nc.tensor.transpose(
                            tps, st["x"][:, hi * 128 : (hi + 1) * 128], ident[:]
                        )
                        nc.scalar.copy(
                            st["xT"][:, hi * 128 : (hi + 1) * 128], tps[:]
                        )

                    pieces.append(p_tr)

                def p_mm():
                    st["m"] = ps_m.tile([128, 256], f32, tag="m", name=f"mch{c}")
                    for hi in range(8):
                        nc.tensor.matmul(
                            st["m"][:],
                            st["xT"][:, hi * 128 : (hi + 1) * 128],
                            bm_sb[:, hi * 256 : (hi + 1) * 256],
                            start=(hi == 0),
                            stop=(hi == 7),
                        )

                pieces.append(p_mm)

                def p_inj():
                    mag = wpool.tile([128, 256], f32, tag="mag")
                    nc.scalar.activation(mag[:], st["m"][:], Act.Abs)
                    mx8 = wpool.tile([128, 8], f32, tag="mx8")
                    nc.vector.max(mx8[:], mag[:])
                    inj = wpool.tile([128, 256], f32, tag="inj")
                    nc.vector.scalar_tensor_tensor(
                        out=inj[:], in0=mag[:], scalar=mx8[:, 7:8], in1=st["m"][:],
                        op0=Alu.is_ge, op1=Alu.mult,
                    )
                    st["inj"] = inj

                pieces.append(p_inj)

                def p_injT():
                    injT = wpool.tile([128, 512], f32, tag="injT")
                    nc.gpsimd.memset(injT[:, 256:512], 0.0)
                    for q in range(2):
                        tps = ps_t.tile([128, 128], f32, tag="tps")
                        nc.tensor.transpose(
                            tps, st["inj"][:, q * 128 : (q + 1) * 128], ident[:]
                        )
                        nc.scalar.copy(injT[:, q * 128 : (q + 1) * 128], tps[:])
                    st["injT"] = injT

                pieces.append(p_injT)
                return st, pieces

            def y_pieces(c, xT_sb, z_sb):
                t0 = c * PCH
                st = {}
                pieces = []

                def p_alloc():
                    st["y"] = xpool.tile([128, H], f32, tag="y", name=f"ych{c}")

                pieces.append(p_alloc)
                for hi in range(8):
                    def p_y(hi=hi):
                        yT_ps = ps_y.tile([128, 128], f32, tag="yT")
                        nc.tensor.matmul(
                            yT_ps[:],
                            by_sb[:, (hi * 2 + 0) * 128 : (hi * 2 + 1) * 128],
                            z_sb[:, 0:128], start=True, stop=False,
                        )
                        nc.tensor.matmul(
                            yT_ps[:],
                            by_sb[:, (hi * 2 + 1) * 128 : (hi * 2 + 2) * 128],
                            z_sb[:, 128:256], start=False, stop=False,
                        )
                        nc.tensor.matmul(
                            yT_ps[:], ident[:],
                            xT_sb[:, hi * 128 : (hi + 1) * 128],
                            start=False, stop=True,
                        )
                        yT_sb = wpool.tile([128, 128], f32, tag="yTs")
                        nc.scalar.copy(yT_sb[:], yT_ps[:])
                        y_ps = ps_y.tile([128, 128], f32, tag="yT")
                        nc.tensor.transpose(y_ps, yT_sb[:], ident[:])
                        nc.scalar.copy(
                            st["y"][:, hi * 128 : (hi + 1) * 128], y_ps[:]
                        )

                    pieces.append(p_y)

                def p_dma():
                    nc.sync.dma_start(yb[t0 : t0 + PCH, :], st["y"][:])

                pieces.append(p_dma)
                return pieces

            from contextlib import nullcontext

            rep_ctx = tc.For_i(0, loop_reps, 1) if loop_reps > 1 else nullcontext()
            with rep_ctx:
                u1, u2 = v0_sb, v0_sb
                r1 = ones_ap
                gr_fifo = [gp_ap, gp_ap]

                cur, pieces0 = prep_pieces(0)
                for p in pieces0:
                    p()

                prev_xT = None
                prev_z = None
                for c in range(n_chunks):
                    z_sb = wpool.tile([128, 256], f32, tag="z")

                    # schedule: y(c-1) pieces at steps 4..58, prep(c+1)
                    # pieces at steps 64..110 (one piece per slot)
                    sched = {}
                    if prev_z is not None:
                        yp = y_pieces(c - 1, prev_xT, prev_z)
                        for i, p in enumerate(yp):
                            sched.setdefault(4 + 6 * i, []).append(p)
                    if c + 1 < n_chunks:
                        nxt, npieces = prep_pieces(c + 1)
                        for i, p in enumerate(npieces):
                            sched.setdefault(64 + 4 * i, []).append(p)
                    else:
                        nxt = None

                    injT = cur["injT"]
                    for t in range(PCH):
                        inj4 = injT[:, t : 512 : 128]
                        gr2 = gr_fifo.pop(0)
                        P = P_ring[t % pring_n]
                        nc.vector.scalar_tensor_tensor(
                            out=P[:, 0:4], in0=u2[:], scalar=gr2, in1=inj4,
                            op0=Alu.mult, op1=Alu.add,
                        )
                        u = spool.tile([128, 4], f32, tag="u")
                        nc.vector.scalar_tensor_tensor(
                            out=u[:], in0=u1[:], scalar=r1, in1=P[:, 0:4],
                            op0=Alu.mult, op1=Alu.add,
                        )
                        sq2 = spool.tile([128, 2], f32, tag="sq2")
                        d0 = spool.tile([128, 4], f32, tag="d0")
                        nc.vector.scalar_tensor_tensor(
                            out=d0[:], in0=u1[:], scalar=2.0, in1=P[:, 0:4],
                            op0=Alu.mult, op1=Alu.mult, accum_out=sq2[:, 0:1],
                        )
                        d1 = spool.tile([128, 5], f32, tag="d1")
                        nc.vector.scalar_tensor_tensor(
                            out=d1[:], in0=P[:], scalar=1.0, in1=P[:],
                            op0=Alu.mult, op1=Alu.mult, accum_out=sq2[:, 1:2],
                        )
                        sq2r = ps_r.tile([128, 2], f32, tag="ps_b")
                        nc.tensor.matmul(
                            sq2r[:], ones_mat[:], sq2[:], start=True, stop=True
                        )
                        pp1_sb = spool.tile([128, 1], f32, tag="pp1")
                        nc.scalar.copy(pp1_sb[:], sq2r[:, 1:2])
                        r_t = spool.tile([128, 1], f32, tag="r")
                        nc.scalar.activation(
                            r_t[:], sq2r[:, 0:1], Act.Abs_reciprocal_sqrt,
                            scale=r1, bias=pp1_sb[:],
                        )
                        gr_t = spool.tile([128, 1], f32, tag="gr")
                        nc.scalar.activation(gr_t[:], r_t[:], Act.Copy, scale=gp_ap)
                        nc.scalar.activation(
                            z_sb[:, t : 256 : 128], u[:, 0:2], Act.Copy,
                            scale=r_t[:],
                        )
                        u2, u1 = u1, u
                        gr_fifo.append(gr_t[:])
                        r1 = r_t[:]

                        for p in sched.get(t, ()):
                            p()

                    prev_xT = cur["xT"]
                    prev_z = z_sb
                    cur = nxt

                # tail: y-block of the last chunk
                for p in y_pieces(n_chunks - 1, prev_xT, prev_z):
                    p()

    nc.compile()
    return nc


def _build_program_v2(
    n_chunks: int,
    loop_reps: int = 1,
    r_mode: str = "abs_rsqrt",
    z_eng: str = "act",
    u_b_eng: str = "dve",
    p_b_eng: str = "dve",
    gr_eng: str = "dve",
):
    # NOTE: Pool (gpsimd) scalar_tensor_tensor/tensor_scalar ops crash the
    # walrus codegen in this toolchain — only "act"/"dve" engines are safe
    # for the scan body (gpsimd memset is fine).
    """Paired-step scan: two timesteps share one PE partition-reduce and one
    ACT round-trip.  Identity used: u_t = r_{t-1} u_{t-1} + P_t does not
    depend on r_t, so IP/PP inner products for steps (2k, 2k+1) are all
    computable before either step's renorm scalar:

      P_a = gp r_{a-2} u_{a-2} + inj_a ;  u_a = r_{a-1} u_{a-1} + P_a
      P_b = gp r_{a-1} u_{a-1} + inj_b ;  u_b = r_a u_a + P_b
      r_a = rsqrt(2<u_{a-1},P_a> r_{a-1} + ||P_a||^2 + 1)
      r_b = rsqrt(2<u_a,  P_b> r_a     + ||P_b||^2 + 1)

    One [128,4] ones-matmul reduces (2IPa, PPa+1, 2IPb, PPb+1) at once; the
    two rsqrts run back-to-back on ACT (Abs_reciprocal_sqrt, measured 4e-5
    rel err on HW, amplified ~12x by the gamma=0.92 contraction -> ~5e-4,
    well under the 2e-2 gate)."""
    import concourse.bass as bass
    import concourse.bacc as bacc
    import concourse.mybir as mybir
    from concourse.tile import TileContext
    from concourse.masks import make_identity

    f32 = mybir.dt.float32
    Alu = mybir.AluOpType
    Act = mybir.ActivationFunctionType
    Tq = n_chunks * PCH

    from concourse._compat import get_trn_type

    nc = bacc.Bacc(get_trn_type() or "TRN2", target_bir_lowering=False, debug=False)
    xb = nc.declare_dram_parameter("xb", [Tq, H], f32, isOutput=False)
    basis_m = nc.declare_dram_parameter("basis_m", [128, 8 * 256], f32, isOutput=False)
    basis_y = nc.declare_dram_parameter("basis_y", [128, 16 * 128], f32, isOutput=False)
    v0d = nc.declare_dram_parameter("v0", [128, 4], f32, isOutput=False)
    scal = nc.declare_dram_parameter("scal", [128, 3], f32, isOutput=False)
    yb = nc.declare_dram_parameter("yb", [Tq, H], f32, isOutput=True)

    P_ENG = {"pool": "gpsimd", "dve": "vector"}

    with TileContext(nc) as tc:
        with (
            tc.tile_pool(name="const", bufs=1) as cpool,
            tc.tile_pool(name="xio", bufs=4) as xpool,
            tc.tile_pool(name="work", bufs=2) as wpool,
            tc.tile_pool(name="scan", bufs=4) as spool,
            tc.tile_pool(name="ps_t", bufs=2, space="PSUM") as ps_t,
            tc.tile_pool(name="ps_m", bufs=1, space="PSUM") as ps_m,
            tc.tile_pool(name="ps_y", bufs=2, space="PSUM") as ps_y,
            tc.tile_pool(name="ps_r", bufs=2, space="PSUM") as ps_r,
        ):
            bm_sb = cpool.tile([128, 8 * 256], f32, tag="bm")
            nc.sync.dma_start(bm_sb, basis_m[:])
            by_sb = cpool.tile([128, 16 * 128], f32, tag="by")
            nc.sync.dma_start(by_sb, basis_y[:])
            v0_sb = cpool.tile([128, 4], f32, tag="v0")
            nc.sync.dma_start(v0_sb, v0d[:])
            sc_sb = cpool.tile([128, 3], f32, tag="sc")
            nc.sync.dma_start(sc_sb, scal[:])
            ident = cpool.tile([128, 128], f32, tag="ident")
            make_identity(nc, ident[:])
            ones_mat = cpool.tile([128, 128], f32, tag="ones_mat")
            nc.vector.memset(ones_mat[:], 1.0)

            ones_ap = sc_sb[:, 0:1]
            gp_ap = sc_sb[:, 1:2]

            warm_acc = ps_m.tile([128, 1], f32, tag="warm")

            def pe_touch(sb_slice):
                nc.tensor.matmul(
                    warm_acc[:], sb_slice, ident[:, 0:1], start=True, stop=True
                )

            pe_touch(ident[:, 0:128])
            pe_touch(bm_sb[:, 0:128])
            pe_touch(by_sb[:, 0:128])
            wsc = spool.tile([128, 4], f32, tag="wsc")
            nc.vector.tensor_copy(wsc[:], v0_sb[:])
            wsc2 = spool.tile([128, 3], f32, tag="wsc2")
            nc.vector.tensor_copy(wsc2[:], sc_sb[:])
            wsc3 = spool.tile([128, 3], f32, tag="wsc3")
            nc.scalar.copy(wsc3[:], sc_sb[:])
            wsc4 = spool.tile([128, 4], f32, tag="wsc4")
            nc.gpsimd.tensor_copy(wsc4[:], v0_sb[:])
            wsc5 = spool.tile([128, 3], f32, tag="wsc5")
            nc.gpsimd.tensor_copy(wsc5[:], sc_sb[:])

            # P ring: cols 0:4 = P_t, col 4 = sqrt(1/128) so the square-accum
            # carries the +1 of s = 1 + 2 r IP + ||P||^2 through the reduce.
            P_ring = []
            for i in range(4):
                pring_tile = cpool.tile([128, 5], f32, tag=f"Pring{i}")
                P_ring.append(pring_tile)
                nc.vector.memset(pring_tile[:, 4:5], float(np.sqrt(1.0 / 128.0)))

            from contextlib import nullcontext

            rep_ctx = tc.For_i(0, loop_reps, 1) if loop_reps > 1 else nullcontext()
            with rep_ctx:
                u1, u2 = v0_sb, v0_sb
                r1 = ones_ap
                grA, grB = gp_ap, gp_ap
                rp_prev = None
                for c in range(n_chunks):
                    t0 = c * PCH
                    x_sb = xpool.tile([128, H], f32, tag="x")
                    nc.sync.dma_start(x_sb, xb[t0 : t0 + PCH, :])

                    xT_sb = xpool.tile([128, H], f32, tag="xT")
                    pe_touch(x_sb[:, 0:128])
                    for hi in range(8):
                        tps = ps_t.tile([128, 128], f32, tag="tps")
                        nc.tensor.transpose(
                            tps, x_sb[:, hi * 128 : (hi + 1) * 128], ident[:]
                        )
                        nc.scalar.copy(xT_sb[:, hi * 128 : (hi + 1) * 128], tps[:])

                    m_ps = ps_m.tile([128, 256], f32, tag="m")
                    for hi in range(8):
                        nc.tensor.matmul(
                            m_ps[:],
                            xT_sb[:, hi * 128 : (hi + 1) * 128],
                            bm_sb[:, hi * 256 : (hi + 1) * 256],
                            start=(hi == 0),
                            stop=(hi == 7),
                        )

                    mag = wpool.tile([128, 256], f32, tag="mag")
                    nc.scalar.activation(mag[:], m_ps[:], Act.Abs)
                    mx8 = wpool.tile([128, 8], f32, tag="mx8")
                    nc.vector.max(mx8[:], mag[:])
                    inj = wpool.tile([128, 256], f32, tag="inj")
                    nc.vector.scalar_tensor_tensor(
                        out=inj[:], in0=mag[:], scalar=mx8[:, 7:8], in1=m_ps[:],
                        op0=Alu.is_ge, op1=Alu.mult,
                    )

                    injT = wpool.tile([128, 512], f32, tag="injT")
                    nc.gpsimd.memset(injT[:, 256:512], 0.0)
                    for q in range(2):
                        tps = ps_t.tile([128, 128], f32, tag="tps")
                        nc.tensor.transpose(
                            tps, inj[:, q * 128 : (q + 1) * 128], ident[:]
                        )
                        nc.scalar.copy(injT[:, q * 128 : (q + 1) * 128], tps[:])

                    z_sb = wpool.tile([128, 256], f32, tag="z")

                    for k in range(PCH // 2):
                        a = 2 * k
                        inj_a = injT[:, a : 512 : 128]
                        inj_b = injT[:, a + 1 : 512 : 128]
                        Pa = P_ring[a % 4]
                        Pb = P_ring[(a + 1) % 4]
                        sq = spool.tile([128, 4], f32, tag="sq")

                        # DVE: everything feeding the reduce (single PE wait).
                        # gr for this pair is computed here (DVE waits on the
                        # previous pair's r anyway) instead of trailing on ACT
                        # where it would delay this pair's DVE start.
                        if gr_eng == "dve" and rp_prev is not None:
                            grp = spool.tile([128, 2], f32, tag="grp")
                            nc.vector.tensor_scalar_mul(grp[:], rp_prev[:], gp_ap)
                            grA, grB = grp[:, 0:1], grp[:, 1:2]
                        nc.vector.scalar_tensor_tensor(
                            out=Pa[:, 0:4], in0=u2[:], scalar=grA, in1=inj_a,
                            op0=Alu.mult, op1=Alu.add,
                        )
                        d0a = spool.tile([128, 4], f32, tag="d0a")
                        nc.vector.scalar_tensor_tensor(
                            out=d0a[:], in0=u1[:], scalar=2.0, in1=Pa[:, 0:4],
                            op0=Alu.mult, op1=Alu.mult, accum_out=sq[:, 0:1],
                        )
                        d1a = spool.tile([128, 5], f32, tag="d1a")
                        nc.vector.scalar_tensor_tensor(
                            out=d1a[:], in0=Pa[:], scalar=1.0, in1=Pa[:],
                            op0=Alu.mult, op1=Alu.mult, accum_out=sq[:, 1:2],
                        )
                        u_a = spool.tile([128, 4], f32, tag="u")
                        nc.vector.scalar_tensor_tensor(
                            out=u_a[:], in0=u1[:], scalar=r1, in1=Pa[:, 0:4],
                            op0=Alu.mult, op1=Alu.add,
                        )
                        # P_b on pool (no r dependence)
                        p_b_e = getattr(nc, P_ENG[p_b_eng])
                        p_b_e.scalar_tensor_tensor(
                            out=Pb[:, 0:4], in0=u1[:], scalar=grB, in1=inj_b,
                            op0=Alu.mult, op1=Alu.add,
                        )
                        d0b = spool.tile([128, 4], f32, tag="d0b")
                        nc.vector.scalar_tensor_tensor(
                            out=d0b[:], in0=u_a[:], scalar=2.0, in1=Pb[:, 0:4],
                            op0=Alu.mult, op1=Alu.mult, accum_out=sq[:, 2:3],
                        )
                        d1b = spool.tile([128, 5], f32, tag="d1b")
                        nc.vector.scalar_tensor_tensor(
                            out=d1b[:], in0=Pb[:], scalar=1.0, in1=Pb[:],
                            op0=Alu.mult, op1=Alu.mult, accum_out=sq[:, 3:4],
                        )

                        sq4r = ps_r.tile([128, 4], f32, tag="ps_b")
                        nc.tensor.matmul(
                            sq4r[:], ones_mat[:], sq[:], start=True, stop=True
                        )

                        pp = spool.tile([128, 2], f32, tag="pp")
                        nc.scalar.copy(pp[:], sq4r[:, 1:4:2])
                        rp = spool.tile([128, 2], f32, tag="rp")
                        if r_mode == "abs_rsqrt":
                            nc.scalar.activation(
                                rp[:, 0:1], sq4r[:, 0:1], Act.Abs_reciprocal_sqrt,
                                scale=r1, bias=pp[:, 0:1],
                            )
                            nc.scalar.activation(
                                rp[:, 1:2], sq4r[:, 2:3], Act.Abs_reciprocal_sqrt,
                                scale=rp[:, 0:1], bias=pp[:, 1:2],
                            )
                        else:
                            n_a = spool.tile([128, 2], f32, tag="n")
                            nc.scalar.activation(
                                n_a[:, 0:1], sq4r[:, 0:1], Act.Sqrt,
                                scale=r1, bias=pp[:, 0:1],
                            )
                            nc.vector.reciprocal(rp[:, 0:1], n_a[:, 0:1])
                            nc.scalar.activation(
                                n_a[:, 1:2], sq4r[:, 2:3], Act.Sqrt,
                                scale=rp[:, 0:1], bias=pp[:, 1:2],
                            )
                            nc.vector.reciprocal(rp[:, 1:2], n_a[:, 1:2])

                        u_b = spool.tile([128, 4], f32, tag="u")
                        u_b_e = getattr(nc, P_ENG[u_b_eng])
                        u_b_e.scalar_tensor_tensor(
                            out=u_b[:], in0=u_a[:], scalar=rp[:, 0:1],
                            in1=Pb[:, 0:4], op0=Alu.mult, op1=Alu.add,
                        )
                        if gr_eng == "act":
                            grp = spool.tile([128, 2], f32, tag="grp")
                            nc.scalar.activation(
                                grp[:], rp[:], Act.Copy, scale=gp_ap
                            )
                            grA, grB = grp[:, 0:1], grp[:, 1:2]
                        elif gr_eng == "pool":
                            grp = spool.tile([128, 2], f32, tag="grp")
                            nc.gpsimd.tensor_scalar_mul(grp[:], rp[:], gp_ap)
                            grA, grB = grp[:, 0:1], grp[:, 1:2]
                        if z_eng == "act":
                            nc.scalar.activation(
                                z_sb[:, a : 256 : 128], u_a[:, 0:2], Act.Copy,
                                scale=rp[:, 0:1],
                            )
                            nc.scalar.activation(
                                z_sb[:, a + 1 : 256 : 128], u_b[:, 0:2], Act.Copy,
                                scale=rp[:, 1:2],
                            )
                        else:
                            z_e = getattr(nc, P_ENG[z_eng])
                            z_e.tensor_scalar_mul(
                                z_sb[:, a : 256 : 128], u_a[:, 0:2], rp[:, 0:1]
                            )
                            z_e.tensor_scalar_mul(
                                z_sb[:, a + 1 : 256 : 128], u_b[:, 0:2], rp[:, 1:2]
                            )

                        u2, u1 = u_a, u_b
                        r1 = rp[:, 1:2]
                        rp_prev = rp

                    y_sb = xpool.tile([128, H], f32, tag="y")
                    for hi in range(8):
                        yT_ps = ps_y.tile([128, 128], f32, tag="yT")
                        nc.tensor.matmul(
                            yT_ps[:], by_sb[:, (hi * 2 + 0) * 128 : (hi * 2 + 1) * 128],
                            z_sb[:, 0:128], start=True, stop=False,
                        )
                        nc.tensor.matmul(
                            yT_ps[:], by_sb[:, (hi * 2 + 1) * 128 : (hi * 2 + 2) * 128],
                            z_sb[:, 128:256], start=False, stop=False,
                        )
                        nc.tensor.matmul(
                            yT_ps[:], ident[:], xT_sb[:, hi * 128 : (hi + 1) * 128],
                            start=False, stop=True,
                        )
                        yT_sb = wpool.tile([128, 128], f32, tag="yTs")
                        nc.scalar.copy(yT_sb[:], yT_ps[:])
                        y_ps = ps_y.tile([128, 128], f32, tag="yT")
                        nc.tensor.transpose(y_ps, yT_sb[:], ident[:])
                        nc.scalar.copy(y_sb[:, hi * 128 : (hi + 1) * 128], y_ps[:])
                    nc.sync.dma_start(yb[t0 : t0 + PCH, :], y_sb[:])

    nc.compile()
    return nc


def _host_pack(inputs):
    """Fold all small parameters host-side; returns per-core constant arrays."""
    basis = np.asarray(inputs["basis"], np.float32)
    alpha = float(np.asarray(inputs["alpha"]))
    w_r = np.asarray(inputs["w_r"], np.float32)
    bg = _sigmoid(np.asarray(inputs["breadth_gate"], np.float32))

    g = _sigmoid(w_r)
    assert np.all(g[:MEM] == g[0]), "vector w_r gate not supported by fast path"
    gp = float(g[0]) / GAMMA

    Wm = (basis[:, :MEM] * (BETA / GAMMA)).astype(np.float32)  # [H, 256]
    Wy = (basis[:, :MEM] * (alpha * bg[None, :MEM])).astype(np.float32)

    # basis_m blocks: block hi = Wm[hi*128:(hi+1)*128, :]  -> cols [hi*256, ...)
    basis_m = np.concatenate(
        [Wm[hi * 128 : (hi + 1) * 128, :] for hi in range(8)], axis=1
    ).astype(np.float32)  # [128, 2048]
    WyT = np.ascontiguousarray(Wy.T)  # [256, 1024]
    blocks = []
    for hi in range(8):
        for q in range(2):
            blocks.append(WyT[q * 128 : (q + 1) * 128, hi * 128 : (hi + 1) * 128])
    basis_y = np.concatenate(blocks, axis=1).astype(np.float32)  # [128, 2048]

    t0c = (
        np.asarray(inputs["tape_init_re"], np.float32)
        + 1j * np.asarray(inputs["tape_init_im"], np.float32)
    )[:MEM].astype(np.complex64)
    nrm = np.float32(np.sqrt(max(float((np.abs(t0c) ** 2).sum(dtype=np.float32)), 1e-16)))
    v0c = (t0c / nrm).astype(np.complex64)
    v0 = np.stack(
        [v0c.real[:128], v0c.real[128:], v0c.imag[:128], v0c.imag[128:]], axis=1
    ).astype(np.float32)  # [128, 4]

    scal = np.empty((128, 3), np.float32)
    scal[:, 0] = 1.0
    scal[:, 1] = gp
    scal[:, 2] = 1.0
    return basis_m, basis_y, v0, scal


def _fast_path_ok(inputs):
    z = lambda k: np.all(np.asarray(inputs[k]) == 0)
    g = _sigmoid(np.asarray(inputs["w_r"], np.float32))
    return (
        z("torque_rotation")
        and z("epsilon_scale")
        and z("epsilon_diag")
        and z("pred_scale")
        and z("pred_diag")
        and bool(np.all(g[:MEM] == g[0]))
    )


def _numpy_fallback(inputs):
    """General-case reference implementation (host). Only used if the inputs
    violate the fast-path structure (never the case for this problem's
    generator); keeps kernel() total."""
    import jax

    with jax.default_device(jax.devices("cpu")[0]):
        import jax.numpy as jnp
        from jax import lax

        x = jnp.asarray(inputs["x"])
        basis = jnp.asarray(inputs["basis"])
        active = jnp.arange(S) < MEM
        amf = active.astype(jnp.float32)
        eta = jax.nn.softplus(jnp.asarray(inputs["eta_raw"]))
        eps = (jnp.asarray(inputs["epsilon_factor"]) * jnp.asarray(inputs["epsilon_scale"])) @ jnp.asarray(
            inputs["epsilon_factor"]).T + jnp.diag(jnp.asarray(inputs["epsilon_diag"]))
        wp = (jnp.asarray(inputs["pred_factor"]) * jnp.asarray(inputs["pred_scale"])) @ jnp.asarray(
            inputs["pred_factor"]).T + jnp.diag(jnp.asarray(inputs["pred_diag"]))
        eps_c = eps.astype(jnp.complex64)
        wp_c = wp.astype(jnp.complex64)
        rot = jnp.exp(1j * jnp.asarray(inputs["torque_rotation"]).astype(jnp.complex64))
        wr_gate = jax.nn.sigmoid(jnp.asarray(inputs["w_r"]))
        bg = jax.nn.sigmoid(jnp.asarray(inputs["breadth_gate"]))
        alpha = jnp.asarray(inputs["alpha"])

        def renorm(tape):
            masked = tape * amf
            nrm = jnp.sqrt(jnp.maximum((jnp.abs(masked) ** 2).sum(-1, keepdims=True), 1e-16))
            return masked / nrm

        tape0 = (jnp.asarray(inputs["tape_init_re"]) + 1j * jnp.asarray(inputs["tape_init_im"])) * amf
        tape0 = renorm(jnp.broadcast_to(tape0, (B, S)))

        def step(carry, x_t):
            tape, prev = carry
            m = jnp.einsum("hs,bh->bs", basis, x_t)
            mag = jnp.abs(m) * amf
            kth = lax.top_k(mag, TOPK)[0][:, -1:]
            injv = jnp.where((mag >= kth) & active, m, 0.0).astype(jnp.complex64)
            rotated = tape * rot
            drive = jnp.einsum("st,bt->bs", eps_c, rotated)
            pred = jnp.einsum("st,bt->bs", wp_c, rotated)
            new = (GAMMA * rotated + eta * drive + BETA * injv + PTS * 1j * pred + wr_gate * prev)
            new = renorm(new)
            y = x_t + alpha * jnp.einsum("hs,bs->bh", basis, bg * new.real)
            return (new, tape), y

        (_, _), ys = lax.scan(step, (tape0, tape0), jnp.swapaxes(x, 0, 1))
        return np.asarray(jnp.swapaxes(ys, 0, 1))


USE_V2 = False
BEST_KW = {"use_rsqrt": True}


def _timing_build(n_chunks: int, loop_reps: int = 1):
    """Builder used by kernel() and test.py's repetition timer."""
    build = _build_program_v2 if USE_V2 else _build_program
    return build(n_chunks, loop_reps=loop_reps, **BEST_KW)


def kernel(n_chunks: int = T // PCH, _want_trace: bool = False, **inputs) -> np.ndarray:
    from concourse.bass_utils import run_bass_kernel_spmd

    x = np.ascontiguousarray(np.asarray(inputs["x"], np.float32))
    assert x.shape == (B, T, H)

    if not _fast_path_ok(inputs):
        return _numpy_fallback(inputs)

    basis_m, basis_y, v0, scal = _host_pack(inputs)

    key = (n_chunks, USE_V2, tuple(sorted(BEST_KW.items())))
    if key not in _program_cache:
        _program_cache[key] = _timing_build(n_chunks)
    nc = _program_cache[key]

    Tq = n_chunks * PCH
    core_ids = list(range(B))
    in_maps = [
        {
            "xb": np.ascontiguousarray(x[b, :Tq]),
            "basis_m": basis_m,
            "basis_y": basis_y,
            "v0": v0,
            "scal": scal,
        }
        for b in core_ids
    ]
    res = run_bass_kernel_spmd(nc, in_maps, core_ids, trace=_want_trace)
    out = np.empty((B, Tq, H), np.float32)
    for b in core_ids:
        out[b] = res.results[b]["yb"]
    if _want_trace:
        kernel._last_results = res
    return out

